# revision 22
# baseline (speedup 1.0000x reference)
"""DCN (cross+deep) Trainium2 Bass kernel, 8 NeuronCores.

Sharding: data-parallel over batch (2048 rows/core); embedding gather +
fp8 quantization + layout done host-side; cross/deep weights replicated.

Per-core dataflow (batch in 4 chunks of 512):
  deep:  fp8e4m3 DoubleRow matmuls (2 k-tiles per instruction, 0.5
         cyc/row) for all 3 layers; x scaled by s_x=128, weights by
         s_w=32, hidden activations re-quantized to fp8 (s_y=128) by the
         relu stage (DVE tensor_scalar mult+max for L0 with bias folded
         into a constant x column; ACT scale+bias+relu for L1/L2);
         y2 kept bf16.
  cross: collapsed algebraically. y_i = yhat_i + C_i with yhat_i =
         x0 * tau_i (per-row scalar), tau_{i+1} = tau_i*(S_i+1) + sig_i,
         S_i = cross_w_i . x0, sig_i = C_i*sum(w_i). Output-layer cross
         part = tau_3 * P with P = out_w_cross . x0. The 4 dots are
         computed as stationary-x matmuls (lhsT = x block [128d,128b],
         rhs = packed weights [128,4]) -- out free size 4, so nearly
         free on PE. Dot precision is recovered with a 3-product hi/lo
         fp8 decomposition: xh@cwh + xh@cwl + xl@cwh2 where
         xl = fp8((x*s_x - xh)*16), cwl = fp8(cw*s_cw - cwh),
         cwh2 = fp8(cw*s_cw/16).
  out:   po = ow_deep . y2 (bf16 matvec, out free 1); recursion + final
         combine on DVE over [128,4] tiles (partition = batch%128,
         free = batch-block); result [128,4] f32 scatter-DMA'd to
         out[2048,1].
"""

import numpy as np
import ml_dtypes
from contextlib import ExitStack

import concourse.tile as tile
import concourse.mybir as mybir
from concourse import bacc
from concourse.bass_utils import run_bass_kernel_spmd

# ---- problem constants (hardcoded; kernel.py must be self-contained) ----
B, F, E = 16384, 26, 32
NF = 1_000_000
D = F * E                    # 832
DEEP = (1024, 512, 256)
N_CROSS = 3
N_CORES = 8
S = B // N_CORES             # 2048 batch rows per core
DP = 1024                    # x padded to 8 k-planes (bias col at 832)
KP = DP // 128               # 8 x k-planes
KL = 7                       # x-lo planes (real dims only)
CHUNK = 512
NCHUNK = S // CHUNK          # 4
NBLK = CHUNK // 128          # 4 batch blocks per chunk
M0, M1, M2 = DEEP[0] // 128, DEEP[1] // 128, DEEP[2] // 128  # 8, 4, 2
NG = 3 * KL                  # dot matmul count per block (hi,wl,lo)

# scales (powers of two)
S_X = 128.0
S_W = 32.0
S_CW = 32.0
S_Y = 128.0
D0 = S_Y / (S_X * S_W)       # L0 psum -> y0*S_Y
D1 = S_Y / (S_Y * S_W)       # L1 psum -> y1*S_Y
D2 = 1.0 / (S_Y * S_W)       # L2 psum -> y2 (unscaled)
DD = 1.0 / (S_X * S_CW)      # dots descale

_bf = mybir.dt.bfloat16
_f8 = mybir.dt.float8e4
_f32 = mybir.dt.float32
_np_bf = ml_dtypes.bfloat16
_np_f8 = ml_dtypes.float8_e4m3

_CACHE = {}
# tuning knobs
CFG = dict(
    act_l0=(1,),   # L0 relu tiles on ACT
    pool_l0=(),    # L0 relu tiles on Pool/GPSIMD (rest on DVE); Pool+PSUM
                   # is rejected by the bir verifier, keep empty
    rec_pool=True,  # run the tau recursion on Pool/GPSIMD
    warm=4,       # PE warm-up matmuls
    dps=4, ddp=2, pop=2,   # PSUM pool depths (banks)
    yp=3, cp=2,
    l1_wlo=False, l2_wlo=False,  # optional weight-lo products (accuracy)
    l0_wlo=False,
)


def _build_nc():
    AF = mybir.ActivationFunctionType
    OP = mybir.AluOpType
    DR = mybir.MatmulPerfMode.DoubleRow
    nc = bacc.Bacc(
        "TRN2", target_bir_lowering=False, debug=False, num_devices=N_CORES
    )

    # x-hi fp8, k-plane-major: xh[p, k*S + b] = x_hi[b, k*128+p]
    xh_d = nc.dram_tensor("xh", [128, KP * S], _f8, kind="ExternalInput")
    # x-lo fp8 (7 real planes): xl[p, k*S + b]
    xl_d = nc.dram_tensor("xl", [128, KL * S], _f8, kind="ExternalInput")
    # deep weights, m-major DR layout: w[p, (m*KK + plane)*128 + c]
    nw0 = 2 if CFG["l0_wlo"] else 1
    nw1 = 2 if CFG["l1_wlo"] else 1
    nw2 = 2 if CFG["l2_wlo"] else 1
    w0_d = nc.dram_tensor("w0", [128, nw0 * M0 * KP * 128], _f8, kind="ExternalInput")
    w1_d = nc.dram_tensor("w1", [128, nw1 * M1 * M0 * 128], _f8, kind="ExternalInput")
    w2_d = nc.dram_tensor("w2", [128, nw2 * M2 * M1 * 128], _f8, kind="ExternalInput")
    # packed cross/out dot weights: cwd[p, g*4 + q], g: 7 hi | 7 wl | 7 lo
    cwd_d = nc.dram_tensor("cwd", [128, NG * 4], _f8, kind="ExternalInput")
    # deep out weights: owd[p, t] = ow[832 + t*128 + p]
    owd_d = nc.dram_tensor("owd", [128, M2], _bf, kind="ExternalInput")
    # f32 constants: [b1r(4) | b2r(2) | sig1 | sig2 | obp] = 9 cols
    cst_d = nc.dram_tensor("cst", [128, M1 + M2 + 3], _f32, kind="ExternalInput")
    # out-bias row for the po bias matmul: [obp x128 | 1.0 x4]
    obc_d = nc.dram_tensor("obc", [1, 132], _f32, kind="ExternalInput")
    out_d = nc.dram_tensor("out", [S, 1], _f32, kind="ExternalOutput")

    with ExitStack() as ctx:
        tc = ctx.enter_context(tile.TileContext(nc))
        wp = ctx.enter_context(tc.tile_pool(name="wp", bufs=1))
        yp = ctx.enter_context(tc.tile_pool(name="yp", bufs=CFG["yp"]))
        cp = ctx.enter_context(tc.tile_pool(name="cp", bufs=CFG["cp"]))
        otp = ctx.enter_context(tc.tile_pool(name="otp", bufs=2))
        dps = ctx.enter_context(tc.tile_pool(name="dps", bufs=CFG["dps"], space="PSUM"))
        ddp = ctx.enter_context(tc.tile_pool(name="ddp", bufs=CFG["ddp"], space="PSUM"))
        pop = ctx.enter_context(tc.tile_pool(name="pop", bufs=CFG["pop"], space="PSUM"))

        # ---- persistent SBUF tensors ----
        cst_sb = wp.tile([128, M1 + M2 + 3], _f32)
        nc.sync.dma_start(cst_sb[:], cst_d[:, :])
        b1_sb = cst_sb[:, 0:M1]
        b2_sb = cst_sb[:, M1:M1 + M2]
        sig1_sb = cst_sb[:, M1 + M2:M1 + M2 + 1]
        sig2_sb = cst_sb[:, M1 + M2 + 1:M1 + M2 + 2]
        obp_sb = cst_sb[:, M1 + M2 + 2:M1 + M2 + 3]
        cwd_sb = wp.tile([128, NG * 4], _f8)
        nc.sync.dma_start(cwd_sb[:], cwd_d[:, :])
        owd_sb = wp.tile([128, M2], _bf)
        nc.sync.dma_start(owd_sb[:], owd_d[:, :])
        obc_sb = wp.tile([1, 132], _f32)
        nc.sync.dma_start(obc_sb[:], obc_d[:, :])

        xh_sb = wp.tile([128, KP, S], _f8)
        xl_sb = wp.tile([128, KL, S], _f8)
        w0_sb = wp.tile([128, nw0 * M0, KP, 128], _f8)
        w1_sb = wp.tile([128, nw1 * M1, M0, 128], _f8)
        w2_sb = wp.tile([128, nw2 * M2, M1, 128], _f8)

        xh_r = xh_d[:, :].rearrange("p (k b) -> p k b", k=KP)
        xl_r = xl_d[:, :].rearrange("p (k b) -> p k b", k=KL)
        w0_r = w0_d[:, :].rearrange("p (m k c) -> p m k c", m=nw0 * M0, k=KP)
        w1_r = w1_d[:, :].rearrange("p (m k c) -> p m k c", m=nw1 * M1, k=M0)
        w2_r = w2_d[:, :].rearrange("p (m k c) -> p m k c", m=nw2 * M2, k=M1)

        def _xh_load(c):
            nc.sync.dma_start(
                xh_sb[:, :, c * CHUNK:(c + 1) * CHUNK],
                xh_r[:, :, c * CHUNK:(c + 1) * CHUNK],
            )

        def _xl_load(c):
            nc.sync.dma_start(
                xl_sb[:, :, c * CHUNK:(c + 1) * CHUNK],
                xl_r[:, :, c * CHUNK:(c + 1) * CHUNK],
            )

        # DMA order: interleaved so the PE pipeline is never input-starved:
        # xh-c0 + first w0 tiles feed L0-c0, xh-c1 lands before L0-c1, w1
        # before L1-c0, xl-c0 before dots-c0. Order == DMA service order.
        def _w0_load(ms):
            for m in ms:
                nc.sync.dma_start(w0_sb[:, m, :, :], w0_r[:, m, :, :])

        def _w1_load(ms):
            for m in ms:
                nc.sync.dma_start(w1_sb[:, m, :, :], w1_r[:, m, :, :])

        _xh_load(0)
        _w0_load(range(0, 4))
        _xh_load(1)
        _w0_load(range(4, nw0 * M0))
        _w1_load(range(0, 2))
        _xh_load(2)
        _w1_load(range(2, nw1 * M1))
        _xl_load(0)
        nc.sync.dma_start(w2_sb[:], w2_r[:, :, :, :])
        _xh_load(3)
        _xl_load(1)
        _xl_load(2)
        _xl_load(3)

        # "Observe" ops: each engine touches its DMA-loaded constants once so
        # steady-state instructions carry at most one semaphore wait.
        obs = wp.tile([128, 8], _f32)
        nc.vector.tensor_copy(obs[:, 0:1], sig1_sb)
        nc.scalar.activation(obs[:, 1:2], b1_sb[:, 0:1], AF.Copy)
        nc.scalar.activation(obs[:, 2:3], b2_sb[:, 0:1], AF.Copy)
        nc.scalar.activation(obs[:, 3:4], obp_sb, AF.Copy)
        nc.vector.tensor_copy(obs[:, 4:5], sig2_sb)

        # PE warm-up: keep PE busy during the startup DMA window.
        warm = wp.tile([128, 512], _bf)
        nc.vector.memset(warm[:], 0.0)
        warm_ps = dps.tile([128, 512], _f32, tag="dps", name="warm_ps")
        for _ in range(CFG["warm"]):
            nc.tensor.matmul(
                warm_ps[:], lhsT=warm[:, 0:128], rhs=warm[:], start=True, stop=True
            )

        def _observe(w_ap, name):
            # weight observe (single-wait rule): tiny matmul into warm_ps,
            # emitted right before the weight's first real use so it doesn't
            # stall the in-order PE queue on late DMAs.
            nc.tensor.matmul(
                warm_ps[0:1, 0:1], lhsT=w_ap, rhs=w_ap, start=True, stop=True
            )

        _observe(cwd_sb[:, 0:1], "cwd")
        _observe(w0_sb[:, 0, 0, 0:1], "w0")

        # per-chunk state carried between pipeline stages
        y0s, y1s, y2s, pps_s, dds_s = {}, {}, {}, {}, {}

        def emit_l0(c):
            cs = slice(c * CHUNK, (c + 1) * CHUNK)
            y0 = yp.tile([128, M0 // 2, 2, CHUNK], _f8, tag="y0", name=f"y0_{c}")
            y0s[c] = y0
            for m in range(M0):
                ps = dps.tile([128, CHUNK], _f32, tag="dps", name=f"ps0_{c}_{m}")
                np_ = KP // 2
                tot = nw0 * np_
                for g in range(tot):
                    wm = m if g < np_ else M0 + m
                    j = g % np_
                    nc.tensor.matmul(
                        ps[:],
                        lhsT=w0_sb[:, wm, 2 * j:2 * j + 2, :],
                        rhs=xh_sb[:, 2 * j:2 * j + 2, cs],
                        perf_mode=DR,
                        start=(g == 0),
                        stop=(g == tot - 1),
                        skip_group_check=True,
                    )
                dst = y0[:, m // 2, m % 2, :]
                if m in CFG["act_l0"]:
                    nc.scalar.activation(dst, ps[:], AF.Relu, bias=0.0, scale=D0)
                elif m in CFG["pool_l0"]:
                    nc.gpsimd.tensor_scalar(
                        out=dst, in0=ps[:], scalar1=D0, scalar2=0.0,
                        op0=OP.mult, op1=OP.max,
                    )
                else:
                    nc.vector.tensor_scalar(
                        out=dst, in0=ps[:], scalar1=D0, scalar2=0.0,
                        op0=OP.mult, op1=OP.max,
                    )

        def emit_l1(c):
            y0 = y0s[c]
            y1 = yp.tile([128, M1 // 2, 2, CHUNK], _f8, tag="y1", name=f"y1_{c}")
            y1s[c] = y1
            for m in range(M1):
                ps = dps.tile([128, CHUNK], _f32, tag="dps", name=f"ps1_{c}_{m}")
                np_ = M0 // 2
                tot = nw1 * np_
                for g in range(tot):
                    wm = m if g < np_ else M1 + m
                    j = g % np_
                    nc.tensor.matmul(
                        ps[:],
                        lhsT=w1_sb[:, wm, 2 * j:2 * j + 2, :],
                        rhs=y0[:, j, :, :],
                        perf_mode=DR,
                        start=(g == 0),
                        stop=(g == tot - 1),
                        skip_group_check=True,
                    )
                nc.scalar.activation(
                    y1[:, m // 2, m % 2, :], ps[:], AF.Relu,
                    bias=b1_sb[:, m:m + 1], scale=D1,
                )

        def emit_l2(c):
            y1 = y1s[c]
            y2 = yp.tile([128, M2, CHUNK], _bf, tag="y2", name=f"y2_{c}")
            y2s[c] = y2
            for m in range(M2):
                ps = dps.tile([128, CHUNK], _f32, tag="dps", name=f"ps2_{c}_{m}")
                np_ = M1 // 2
                tot = nw2 * np_
                for g in range(tot):
                    wm = m if g < np_ else M2 + m
                    j = g % np_
                    nc.tensor.matmul(
                        ps[:],
                        lhsT=w2_sb[:, wm, 2 * j:2 * j + 2, :],
                        rhs=y1[:, j, :, :],
                        perf_mode=DR,
                        start=(g == 0),
                        stop=(g == tot - 1),
                        skip_group_check=True,
                    )
                nc.scalar.activation(
                    y2[:, m, :], ps[:], AF.Relu,
                    bias=b2_sb[:, m:m + 1], scale=D2,
                )

        def emit_dots(c):
            dds = ddp.tile([128, 512], _f32, tag="ddp", name=f"dd_{c}")
            dds_s[c] = dds
            ddv = dds[:, 0:16].rearrange("p (j q) -> p j q", q=4)
            for jb in range(NBLK):
                bs = slice(c * CHUNK + jb * 128, c * CHUNK + (jb + 1) * 128)
                for g in range(NG):
                    k = g % KL
                    src = xh_sb if g < 2 * KL else xl_sb
                    nc.tensor.matmul(
                        ddv[:, jb, :],
                        lhsT=src[:, k, bs],
                        rhs=cwd_sb[:, g * 4:(g + 1) * 4],
                        start=(jb == 0 and g == 0),
                        stop=(jb == NBLK - 1 and g == NG - 1),
                        skip_group_check=True,
                    )

        def emit_po(c):
            y2 = y2s[c]
            pps = pop.tile([128, 512], _f32, tag="pop", name=f"po_{c}")
            pps_s[c] = pps
            for jb in range(NBLK):
                lbs = slice(jb * 128, (jb + 1) * 128)
                for t in range(M2):
                    nc.tensor.matmul(
                        pps[:, jb:jb + 1],
                        lhsT=y2[:, t, lbs],
                        rhs=owd_sb[:, t:t + 1],
                        start=(jb == 0 and t == 0),
                        stop=False,
                        skip_group_check=True,
                    )
            # fold the output constant (out_b + C3*sum(ow_cross)) into po
            # via a K=1 f32 bias matmul
            nc.tensor.matmul(
                pps[:, 0:4],
                lhsT=obc_sb[0:1, 0:128],
                rhs=obc_sb[0:1, 128:132],
                start=False,
                stop=True,
                skip_group_check=True,
            )

        rec_s = {}

        rec_eng = nc.gpsimd if CFG["rec_pool"] else nc.vector

        def emit_rec_a(c):
            # tau chain: needs only the dots psum (S0,S1,S2,P). Pool cannot
            # touch PSUM, so stage the 16 dot values to SBUF via ACT first.
            dds = dds_s[c]
            stg = cp.tile([128, 16], _f32, tag="stg", name=f"stg_{c}")
            nc.scalar.activation(stg[:], dds[:, 0:16], AF.Copy)
            dq = stg[:].rearrange("p (j q) -> p q j", q=4)
            t1 = cp.tile([128, 4], _f32, tag="t1", name=f"t1_{c}")
            u1 = cp.tile([128, 4], _f32, tag="u1", name=f"u1_{c}")
            u2 = cp.tile([128, 4], _f32, tag="u2", name=f"u2_{c}")
            rec_eng.tensor_scalar(
                out=t1[:], in0=dq[:, 0, :], scalar1=DD, scalar2=1.0,
                op0=OP.mult, op1=OP.add,
            )
            rec_eng.tensor_scalar(
                out=u1[:], in0=dq[:, 1, :], scalar1=DD, scalar2=1.0,
                op0=OP.mult, op1=OP.add,
            )
            rec_eng.tensor_scalar(
                out=u2[:], in0=dq[:, 2, :], scalar1=DD, scalar2=1.0,
                op0=OP.mult, op1=OP.add,
            )
            ta = cp.tile([128, 4], _f32, tag="ta", name=f"ta_{c}")
            rec_eng.tensor_tensor(out=ta[:], in0=t1[:], in1=u1[:], op=OP.mult)
            tb = cp.tile([128, 4], _f32, tag="tb", name=f"tb_{c}")
            rec_eng.tensor_scalar(
                out=tb[:], in0=ta[:], scalar1=sig1_sb, scalar2=None, op0=OP.add
            )
            tc_ = cp.tile([128, 4], _f32, tag="tc", name=f"tc_{c}")
            rec_eng.tensor_tensor(out=tc_[:], in0=tb[:], in1=u2[:], op=OP.mult)
            t3 = cp.tile([128, 4], _f32, tag="t3", name=f"t3_{c}")
            rec_eng.tensor_scalar(
                out=t3[:], in0=tc_[:], scalar1=sig2_sb, scalar2=None, op0=OP.add
            )
            tq = cp.tile([128, 4], _f32, tag="tq", name=f"tq_{c}")
            rec_eng.tensor_tensor(out=tq[:], in0=t3[:], in1=dq[:, 3, :], op=OP.mult)
            rec_s[c] = tq

        def emit_rec_b(c):
            # final combine: out = tq*DD + (po + obp); reads PSUM -> DVE
            cs = slice(c * CHUNK, (c + 1) * CHUNK)
            tq, pps = rec_s[c], pps_s[c]
            ot = otp.tile([128, 4], _f32, tag="ot", name=f"ot_{c}")
            nc.vector.scalar_tensor_tensor(
                out=ot[:], in0=tq[:], scalar=DD, in1=pps[:, 0:4],
                op0=OP.mult, op1=OP.add,
            )
            nc.sync.dma_start(
                out=out_d[cs, :].rearrange("(j p) o -> p (j o)", p=128),
                in_=ot[:],
            )

        # Software-pipelined emission: L1/dots lag L0 by one chunk,
        # L2/po/final-combine by two, so the in-order PE stream never waits
        # on a relu of the same chunk.
        for st in range(NCHUNK + 2):
            if st < NCHUNK:
                emit_l0(st)
            if 1 <= st < NCHUNK + 1:
                c = st - 1
                if c == 0:
                    _observe(w1_sb[:, 0, 0, 0:1], "w1")
                emit_l1(c)
                emit_dots(c)
                emit_rec_a(c)
            if st >= 2:
                c = st - 2
                if c == 0:
                    _observe(w2_sb[:, 0, 0, 0:1], "w2")
                    _observe(owd_sb[:, 0:1], "owd")
                emit_l2(c)
                emit_po(c)
                emit_rec_b(c)

    nc.compile()
    return nc


def _get_nc():
    if "nc" not in _CACHE:
        _CACHE["nc"] = _build_nc()
    return _CACHE["nc"]


def _q8(a):
    return np.asarray(a, dtype=np.float32).astype(_np_f8)


def _prep_in_maps(inputs):
    fi = np.asarray(inputs["feature_index"]).astype(np.int64)
    fvv = np.asarray(inputs["feature_value"], dtype=np.float32)
    emb = np.asarray(inputs["emb_table"], dtype=np.float32)
    cw = np.asarray(inputs["cross_w"], dtype=np.float32)
    cb = np.asarray(inputs["cross_b"], dtype=np.float32)
    w0 = np.asarray(inputs["w0"], dtype=np.float32)
    b0 = np.asarray(inputs["b0"], dtype=np.float32)
    w1 = np.asarray(inputs["w1"], dtype=np.float32)
    b1 = np.asarray(inputs["b1"], dtype=np.float32)
    w2 = np.asarray(inputs["w2"], dtype=np.float32)
    b2 = np.asarray(inputs["b2"], dtype=np.float32)
    ow = np.asarray(inputs["out_w"], dtype=np.float32).reshape(-1)
    ob = np.asarray(inputs["out_b"], dtype=np.float32).reshape(-1)

    with_fv = not bool(np.all(fvv == 1.0))
    # ---- x gather + hi/lo fp8 quantization (host) ----
    if with_fv:
        xg = emb[fi] * fvv[:, :, None]                 # [B,F,E] f32
        xflat = xg.reshape(B, D) * S_X
        xh_all = np.zeros((B, DP), _np_f8)
        xh_all[:, :D] = xflat.astype(_np_f8)
        res = (xflat - xh_all[:, :D].astype(np.float32)) * 16.0
        xl_all = np.zeros((B, KL * 128), _np_f8)
        xl_all[:, :D] = res.astype(_np_f8)
    else:
        th = np.zeros((NF + 1, E), _np_f8)
        tscaled = emb * S_X
        th[:NF] = tscaled.astype(_np_f8)
        tl = np.zeros((NF + 1, E), _np_f8)
        tl[:NF] = ((tscaled - th[:NF].astype(np.float32)) * 16.0).astype(_np_f8)
        idxp = np.full((B, DP // E), NF, dtype=np.int64)
        idxp[:, :F] = fi
        xh_all = th[idxp].reshape(B, DP)
        xl_all = tl[idxp[:, :KL * 128 // E]].reshape(B, KL * 128)
    xh_all[:, D] = np.float32(S_X).astype(_np_f8)      # bias-one column (x=1*S_X)

    # ---- deep weights (m-major DR layout) ----
    nw0 = 2 if CFG["l0_wlo"] else 1
    nw1 = 2 if CFG["l1_wlo"] else 1
    nw2 = 2 if CFG["l2_wlo"] else 1

    def _wlayout(wq8_list, K, M):
        # wq8_list: list of [K*128, M*128] fp8 arrays (hi, optional lo)
        # -> [128, (len*M)*K*128] with m-major order (hi m's then lo m's)
        mats = np.concatenate([a.reshape(K, 128, M, 128) for a in wq8_list], axis=2)
        return np.ascontiguousarray(
            mats.transpose(1, 2, 0, 3).reshape(128, -1)
        )

    w0p = np.zeros((DP, DEEP[0]), np.float32)
    w0p[:D] = w0 * S_W
    w0p[D] = b0 * S_W                                   # bias row
    w0h = w0p.astype(_np_f8)
    w0l = [(w0p - w0h.astype(np.float32)).astype(_np_f8)] if CFG["l0_wlo"] else []
    w0_dr = _wlayout([w0h] + w0l, KP, M0)

    w1s = w1 * S_W
    w1h = w1s.astype(_np_f8)
    w1l = [(w1s - w1h.astype(np.float32)).astype(_np_f8)] if CFG["l1_wlo"] else []
    w1_dr = _wlayout([w1h] + w1l, M0, M1)

    w2s = w2 * S_W
    w2h = w2s.astype(_np_f8)
    w2l = [(w2s - w2h.astype(np.float32)).astype(_np_f8)] if CFG["l2_wlo"] else []
    w2_dr = _wlayout([w2h] + w2l, M1, M2)

    # ---- packed cross/out dot weights ----
    cwp = np.zeros((4, DP), np.float32)
    cwp[:N_CROSS, :D] = cw
    cwp[3, :D] = ow[:D]
    cwh = (cwp * S_CW).astype(_np_f8)
    cwl = (cwp * S_CW - cwh.astype(np.float32)).astype(_np_f8)
    cwh2 = (cwp * S_CW / 16.0).astype(_np_f8)
    cwd = np.zeros((128, NG * 4), _np_f8)
    for g in range(KL):
        cwd[:, g * 4:(g + 1) * 4] = cwh[:, g * 128:(g + 1) * 128].T
        cwd[:, (KL + g) * 4:(KL + g + 1) * 4] = cwl[:, g * 128:(g + 1) * 128].T
        cwd[:, (2 * KL + g) * 4:(2 * KL + g + 1) * 4] = cwh2[:, g * 128:(g + 1) * 128].T

    owd = np.ascontiguousarray(ow[D:].reshape(M2, 128).T.astype(_np_bf))

    # ---- f32 constants ----
    C = np.cumsum(cb)
    b1r = (S_Y * b1).reshape(M1, 128).T.astype(np.float32)
    b2r = b2.reshape(M2, 128).T.astype(np.float32)
    sig1 = np.full((128, 1), C[0] * cw[1].sum(), np.float32)
    sig2 = np.full((128, 1), C[1] * cw[2].sum(), np.float32)
    obp = np.full((128, 1), ob[0] + C[2] * ow[:D].sum(), np.float32)
    cst = np.ascontiguousarray(np.concatenate([b1r, b2r, sig1, sig2, obp], axis=1))
    obc = np.zeros((1, 132), np.float32)
    obc[0, :128] = ob[0] + C[2] * ow[:D].sum()
    obc[0, 128:] = 1.0

    shared = dict(w0=w0_dr, w1=w1_dr, w2=w2_dr, cwd=cwd, owd=owd, cst=cst, obc=obc)

    in_maps = []
    for core in range(N_CORES):
        rows = slice(core * S, (core + 1) * S)
        # [S, K*128] -> [128, K*S]: xdr[p, k*S+b] = x[b, k*128+p]
        xh8 = (
            xh_all[rows].view(np.uint8).reshape(S, KP, 128)
            .transpose(2, 1, 0).reshape(128, KP * S)
        )
        xl8 = (
            xl_all[rows].view(np.uint8).reshape(S, KL, 128)
            .transpose(2, 1, 0).reshape(128, KL * S)
        )
        m = dict(
            xh=np.ascontiguousarray(xh8).view(_np_f8),
            xl=np.ascontiguousarray(xl8).view(_np_f8),
            **shared,
        )
        in_maps.append(m)
    return in_maps


def _run(inputs, trace=False, **kw):
    nc = _get_nc()
    in_maps = _prep_in_maps(inputs)
    res = run_bass_kernel_spmd(
        nc, in_maps, core_ids=list(range(N_CORES)), trace=trace, **kw
    )
    out = np.concatenate([r["out"] for r in res.results], axis=0)
    return out.astype(np.float32), res


def kernel(**inputs) -> np.ndarray:
    out, _ = _run(inputs, trace=False)
    return out
